# revision 26
# baseline (speedup 1.0000x reference)
"""Trainium2 Bass kernel for nn_AdaptivePoolingClassifier.

Math: the reference MLP has no nonlinearity between its first three layers,
so they collapse into one 128x128 matmul:
    h3 = x @ Wc + bc          with Wc = W1@W2@W3, bc = ((b1@W2+b2)@W3+b3)
    p  = relu(h3) @ W4 + b4                       # [N, 5]
    out[n] = sum_r p[r,n]*w[r,n],  w = softmax(alpha*p, axis=rows)

Softmax weights are invariant to a per-component constant shift, so with
raw = relu(h3) @ (W4*alpha)  (no bias):
    w       = softmax(raw)                 (shift alpha*b4 cancels)
    A[j]    = sum_r raw[r,j] * e^{raw[r,j]}
    B[j]    = sum_r e^{raw[r,j]}
    out[j]  = A[j] / (alpha[j] * B[j]) + b4[j]
The kernel only computes per-core partial A and B; the host finishes.

Sharding: rows split across 8 NeuronCores; partials kept per-partition/
per-group on chip ([128, 320] f32 per core), host adds and divides.

Per-core dataflow (bf16 compute, f32 accumulation), per 1024-row tile:
  cast-DMA (f32->bf16, SWDGE, 16KB contiguous per partition) -> x_sb
  PE transpose-mode per 128-row block -> xT (bf16, PSUM) -> DVE evac to SBUF
  PE: h3T = Wc^T @ xT  (two N=512 matmuls)
  relu(h3T + bc): ACT on cols [0, act_cols), DVE on the rest -> SBUF bf16
  PE per 128-row block: q[rows, 5] = relu_blk^T @ W4a  (start/stop per block)
  ACT: e = exp(q); DVE: qe = q*e; acc[e|qe] += (one combined add)
  single DMA out: [128, 320] f32 partials per core.

~34 warmup matmuls on zeroed scratch run during the DMA ramp so the PE HAM
clock-gate reaches 2.4 GHz before real data lands.
"""

import sys
import numpy as np

_REPO = "/opt/trn_rl_repo"
if _REPO not in sys.path:
    sys.path.insert(0, _REPO)

import concourse.bacc as bacc  # noqa: E402
import concourse.mybir as mybir  # noqa: E402
from concourse import tile  # noqa: E402
from concourse.bass_utils import run_bass_kernel_spmd  # noqa: E402

import ml_dtypes  # noqa: E402

BF16 = ml_dtypes.bfloat16

N_CORES = 8
D = 128
NQ = 5  # q = relu(h3) @ (W4*alpha), 5 cols
TILE_ROWS = 1024
BLOCKS_PER_TILE = TILE_ROWS // 128  # 8
CHUNK_TILES = 4  # stats chunk = 4096 rows
GROUPS_PER_CHUNK = CHUNK_TILES * BLOCKS_PER_TILE  # 32
STATS_W = GROUPS_PER_CHUNK * NQ  # 160
ACT_COLS = 864  # columns of the relu done on ScalarE (rest on DVE)
WARMUP_MMS = 28
TILES_PER_DMA = 4


def build_kernel(rows_per_core: int, act_cols: int = ACT_COLS,
                 warmup: int = WARMUP_MMS):
    """Build the per-core Bacc graph. rows_per_core must divide into chunks."""
    assert rows_per_core % (TILE_ROWS * CHUNK_TILES) == 0
    n_chunks = rows_per_core // (TILE_ROWS * CHUNK_TILES)
    n_tiles = rows_per_core // TILE_ROWS

    f32 = mybir.dt.float32
    bf16 = mybir.dt.bfloat16

    nc = bacc.Bacc("TRN2", target_bir_lowering=False, debug=False,
                   num_devices=N_CORES)

    x_ext = nc.declare_dram_parameter("x", [rows_per_core, D], f32,
                                      isOutput=False)
    # packed bf16 consts: [wc | identity | w4a]
    cb_ext = nc.declare_dram_parameter("cb", [D, 2 * D + NQ], bf16,
                                       isOutput=False)
    bc_ext = nc.declare_dram_parameter("bc", [D, 1], f32, isOutput=False)
    out_ext = nc.declare_dram_parameter("out", [D, 2 * STATS_W], f32,
                                        isOutput=True)

    # DMA granule: partition p holds 32 *consecutive* rows (16 KB contiguous
    # per partition -> 8 KB bf16 write packets); one DMA feeds four compute
    # tiles. Row order within a tile is permuted vs. DRAM, which is fine: the
    # softmax pooling is row-permutation invariant.
    DMA_BLOCKS = TILES_PER_DMA * BLOCKS_PER_TILE
    x_r = x_ext.ap().rearrange("(g p k) f -> g p k f", p=128, k=DMA_BLOCKS)
    n_granules = n_tiles // TILES_PER_DMA

    with tile.TileContext(nc) as tc:
        with (
            tc.tile_pool(name="consts", bufs=1) as cpool,
            tc.tile_pool(name="xf32", bufs=1) as fpool,
            tc.tile_pool(name="xin", bufs=5) as xpool,
            tc.tile_pool(name="xt", bufs=6) as xtpool,
            tc.tile_pool(name="relu", bufs=6) as rpool,
            tc.tile_pool(name="stats", bufs=4) as spool,
            tc.tile_pool(name="acc", bufs=1) as apool,
            tc.tile_pool(name="ps_xt", bufs=2, space="PSUM") as ps_xt,
            tc.tile_pool(name="ps_h3", bufs=2, space="PSUM") as ps_h3,
            tc.tile_pool(name="ps_pq", bufs=2, space="PSUM") as ps_pq,
        ):
            cb_sb = cpool.tile([D, 2 * D + NQ], bf16)
            nc.scalar.dma_start(out=cb_sb[:], in_=cb_ext[:])
            wc_sb = cb_sb[:, 0:D]
            ident_sb = cb_sb[:, D:2 * D]
            w4a_sb = cb_sb[:, 2 * D:2 * D + NQ]
            bc_sb = cpool.tile([D, 1], f32)
            nc.scalar.dma_start(out=bc_sb[:], in_=bc_ext[:])

            # acc layout: [e (160) | qe (160)]
            acc = apool.tile([D, 2 * STATS_W], f32)
            nc.vector.memset(acc[:], 0.0)

            # PE warmup: matmuls on zeroed scratch keep the HAM activity
            # window busy during the DMA ramp so real tiles run at 2.4 GHz.
            if warmup > 0:
                wu_sb = cpool.tile([D, D], bf16)
                nc.vector.memset(wu_sb[:], 0.0)
                # share the pq slot rotation -> no extra PSUM bank
                wu_ps = ps_pq.tile([D, D], f32, tag="pq")
                for _ in range(warmup):
                    nc.tensor.matmul(wu_ps[:], wu_sb[:], wu_sb[:],
                                     start=True, stop=True)

            x_dma = None
            for chunk in range(n_chunks):
                pq = ps_pq.tile([D, STATS_W], f32, tag="pq")
                for t in range(CHUNK_TILES):
                    g_tile = chunk * CHUNK_TILES + t
                    sub = g_tile % TILES_PER_DMA
                    granule = g_tile // TILES_PER_DMA
                    if sub == 0:
                        x_dma = xpool.tile([D, TILES_PER_DMA * TILE_ROWS],
                                           bf16, tag="x_dma")
                        xg = x_r[granule]
                        if granule == 0:
                            # head granule via HWDGE (sync queue) as raw f32
                            # + DVE casts: starts streaming concurrently with
                            # the SWDGE stream and lands tiles earlier
                            xf = fpool.tile([D, TILES_PER_DMA * TILE_ROWS],
                                            f32)
                            for sg in range(TILES_PER_DMA):
                                sl = slice(sg * TILE_ROWS,
                                           (sg + 1) * TILE_ROWS)
                                nc.sync.dma_start(
                                    out=xf[:, sl],
                                    in_=xg[:, sg * BLOCKS_PER_TILE:
                                           (sg + 1) * BLOCKS_PER_TILE],
                                )
                                nc.vector.tensor_copy(x_dma[:, sl],
                                                      xf[:, sl])
                        elif granule <= 2 or granule == n_granules - 1:
                            # per-tile SWDGE cast DMAs: finer arrivals at
                            # ramp head and de-quantized tail
                            for sg in range(TILES_PER_DMA):
                                sl = slice(sg * TILE_ROWS,
                                           (sg + 1) * TILE_ROWS)
                                nc.gpsimd.dma_start(
                                    out=x_dma[:, sl],
                                    in_=xg[:, sg * BLOCKS_PER_TILE:
                                           (sg + 1) * BLOCKS_PER_TILE],
                                )
                        else:
                            # SWDGE cast DMA: f32 HBM -> bf16 SBUF
                            nc.gpsimd.dma_start(out=x_dma[:], in_=xg)
                    x_sb = x_dma[:, sub * TILE_ROWS:(sub + 1) * TILE_ROWS]

                    xt_ps = ps_xt.tile([D, TILE_ROWS], bf16)
                    for k in range(BLOCKS_PER_TILE):
                        nc.tensor.transpose(
                            xt_ps[:, 128 * k:128 * (k + 1)],
                            x_sb[:, 128 * k:128 * (k + 1)],
                            ident_sb[:],
                        )
                    xt_sb = xtpool.tile([D, TILE_ROWS], bf16)
                    nc.vector.tensor_copy(xt_sb[:], xt_ps[:])

                    h3_ps = ps_h3.tile([D, TILE_ROWS], f32)
                    for half in range(TILE_ROWS // 512):
                        nc.tensor.matmul(
                            h3_ps[:, 512 * half:512 * (half + 1)],
                            wc_sb[:],
                            xt_sb[:, 512 * half:512 * (half + 1)],
                            start=True, stop=True)

                    relu_sb = rpool.tile([D, TILE_ROWS], bf16)
                    a = min(act_cols, TILE_ROWS)
                    if a > 0:
                        nc.scalar.activation(
                            relu_sb[:, 0:a], h3_ps[:, 0:a],
                            mybir.ActivationFunctionType.Relu,
                            bias=bc_sb[:, 0:1], scale=1.0,
                        )
                    if a < TILE_ROWS:
                        nc.vector.tensor_scalar(
                            relu_sb[:, a:TILE_ROWS], h3_ps[:, a:TILE_ROWS],
                            bc_sb[:, 0:1], 0.0,
                            mybir.AluOpType.add, mybir.AluOpType.max,
                        )

                    for k in range(BLOCKS_PER_TILE):
                        g = t * BLOCKS_PER_TILE + k
                        nc.tensor.matmul(
                            pq[:, NQ * g:NQ * (g + 1)],
                            relu_sb[:, 128 * k:128 * (k + 1)],
                            w4a_sb[:],
                            start=True, stop=True,
                        )
                # stats: e = exp(q) on ACT, qe = q*e on DVE, acc += [e|qe]
                # on GPSIMD (SBUF-only op, keeps ACT/DVE free for relu/evac).
                # Last chunk: split in halves + add on DVE to shorten the
                # serial drain after the final tile.
                st = spool.tile([D, 2 * STATS_W], f32)
                last = chunk == n_chunks - 1
                HW = STATS_W // 2
                for hf in ([0, 1] if last else [0]):
                    w = HW if last else STATS_W
                    pqh = pq[:, hf * HW:hf * HW + w]
                    eh = st[:, hf * HW:hf * HW + w]
                    qeh = st[:, STATS_W + hf * HW:STATS_W + hf * HW + w]
                    nc.scalar.activation(eh, pqh,
                                         mybir.ActivationFunctionType.Exp)
                    nc.vector.tensor_mul(qeh, pqh, eh)
                    if last:
                        nc.vector.tensor_add(
                            acc[:, hf * HW:hf * HW + w],
                            acc[:, hf * HW:hf * HW + w], eh)
                        nc.vector.tensor_add(
                            acc[:, STATS_W + hf * HW:STATS_W + hf * HW + w],
                            acc[:, STATS_W + hf * HW:STATS_W + hf * HW + w],
                            qeh)
                if not last:
                    nc.gpsimd.tensor_add(acc[:], acc[:], st[:])

            nc.sync.dma_start(out=out_ext[:], in_=acc[:])

    nc.compile()
    return nc


def _prep_consts(W1, b1, W2, b2, W3, b3, W4, b4, alpha):
    Wc = (W1.astype(np.float64) @ W2.astype(np.float64)
          @ W3.astype(np.float64))
    bc = ((b1.astype(np.float64) @ W2.astype(np.float64)
           + b2.astype(np.float64)) @ W3.astype(np.float64)
          + b3.astype(np.float64))
    W4a = W4.astype(np.float64) * alpha.astype(np.float64)[None, :]
    return (
        Wc.astype(BF16),
        bc.astype(np.float32).reshape(D, 1),
        W4a.astype(BF16),
    )


_CACHE = {}


def _get_nc(rows_per_core):
    key = rows_per_core
    if key not in _CACHE:
        _CACHE[key] = build_kernel(rows_per_core)
    return _CACHE[key]


def make_in_maps(x, W1, b1, W2, b2, W3, b3, W4, b4, alpha):
    x = np.asarray(x)
    n_total = x.shape[1]
    rows_per_core = n_total // N_CORES
    wc_bf, bc_f32, w4a_bf = _prep_consts(
        np.asarray(W1), np.asarray(b1), np.asarray(W2), np.asarray(b2),
        np.asarray(W3), np.asarray(b3), np.asarray(W4), np.asarray(b4),
        np.asarray(alpha))
    # packed bf16 const block: [wc | identity | w4a]
    cb = np.zeros((D, 2 * D + NQ), dtype=BF16)
    cb[:, 0:D] = wc_bf
    cb[:, D:2 * D] = np.eye(D, dtype=BF16)
    cb[:, 2 * D:2 * D + NQ] = w4a_bf

    xs = np.ascontiguousarray(x.reshape(n_total, D))
    in_maps = []
    for c in range(N_CORES):
        in_maps.append({
            "x": xs[c * rows_per_core:(c + 1) * rows_per_core],
            "cb": cb,
            "bc": bc_f32,
        })
    return in_maps, rows_per_core


def _reference_host(x, W1, b1, W2, b2, W3, b3, W4, b4, alpha):
    """Exact fallback (used only if some alpha[j] == 0)."""
    x64 = np.asarray(x, np.float64).reshape(-1, D)
    h = x64 @ np.asarray(W1, np.float64) + np.asarray(b1, np.float64)
    h = h @ np.asarray(W2, np.float64) + np.asarray(b2, np.float64)
    h = h @ np.asarray(W3, np.float64) + np.asarray(b3, np.float64)
    h = np.maximum(h, 0.0)
    p = h @ np.asarray(W4, np.float64) + np.asarray(b4, np.float64)
    q = np.asarray(alpha, np.float64) * p
    q -= q.max(axis=0, keepdims=True)
    w = np.exp(q)
    w /= w.sum(axis=0, keepdims=True)
    return (p * w).sum(axis=0)[None, :].astype(np.float32)


def run(inputs, trace=False, **run_kwargs):
    """Run the kernel; returns (full_output, BassKernelResults)."""
    in_maps, rows_per_core = make_in_maps(**inputs)
    nc = _get_nc(rows_per_core)
    try:
        res = run_bass_kernel_spmd(nc, in_maps, list(range(N_CORES)),
                                   trace=trace, **run_kwargs)
    except Exception:
        # one retry for transient device errors
        res = run_bass_kernel_spmd(nc, in_maps, list(range(N_CORES)),
                                   trace=trace, **run_kwargs)
    out = _finish(res.results, np.asarray(inputs["alpha"]),
                  np.asarray(inputs["b4"]))
    return out, res


def kernel(x, W1, b1, W2, b2, W3, b3, W4, b4, alpha):
    alpha = np.asarray(alpha)
    if np.any(alpha == 0.0):
        return _reference_host(x, W1, b1, W2, b2, W3, b3, W4, b4, alpha)
    out, _ = run(dict(x=x, W1=W1, b1=b1, W2=W2, b2=b2, W3=W3, b3=b3,
                      W4=W4, b4=b4, alpha=alpha))
    return out


def _finish(results, alpha, b4):
    S = np.zeros((D, 2 * STATS_W), dtype=np.float64)
    for r in results:
        S += r["out"].astype(np.float64)
    # acc layout: [e (160) | qe (160)], each [groups, 5]
    se = S[:, :STATS_W].reshape(D, GROUPS_PER_CHUNK, NQ).sum(axis=(0, 1))
    sqe = S[:, STATS_W:].reshape(D, GROUPS_PER_CHUNK, NQ).sum(axis=(0, 1))
    out = sqe / (alpha.astype(np.float64) * se) + b4.astype(np.float64)
    return out[None, :].astype(np.float32)


# revision 27
# speedup vs baseline: 1.1211x; 1.1211x over previous
"""Trainium2 Bass kernel for nn_AdaptivePoolingClassifier.

Math: the reference MLP has no nonlinearity between its first three layers,
so they collapse into one 128x128 matmul:
    h3 = x @ Wc + bc          with Wc = W1@W2@W3, bc = ((b1@W2+b2)@W3+b3)
    p  = relu(h3) @ W4 + b4                       # [N, 5]
    out[n] = sum_r p[r,n]*w[r,n],  w = softmax(alpha*p, axis=rows)

Softmax weights are invariant to a per-component constant shift, so with
raw = relu(h3) @ (W4*alpha)  (no bias):
    w       = softmax(raw)                 (shift alpha*b4 cancels)
    A[j]    = sum_r raw[r,j] * e^{raw[r,j]}
    B[j]    = sum_r e^{raw[r,j]}
    out[j]  = A[j] / (alpha[j] * B[j]) + b4[j]
The kernel only computes per-core partial A and B; the host finishes.

Sharding: rows split across 8 NeuronCores; partials kept per-partition/
per-group on chip ([128, 320] f32 per core), host adds and divides.

Per-core dataflow (bf16 compute, f32 accumulation), per 1024-row tile:
  cast-DMA (f32->bf16, SWDGE, 16KB contiguous per partition) -> x_sb
  PE transpose-mode per 128-row block -> xT (bf16, PSUM) -> DVE evac to SBUF
  PE: h3T = Wc^T @ xT  (two N=512 matmuls)
  relu(h3T + bc): ACT on cols [0, act_cols), DVE on the rest -> SBUF bf16
  PE per 128-row block: q[rows, 5] = relu_blk^T @ W4a  (start/stop per block)
  ACT: e = exp(q); DVE: qe = q*e; acc[e|qe] += (one combined add)
  single DMA out: [128, 320] f32 partials per core.

~34 warmup matmuls on zeroed scratch run during the DMA ramp so the PE HAM
clock-gate reaches 2.4 GHz before real data lands.
"""

import sys
import numpy as np

_REPO = "/opt/trn_rl_repo"
if _REPO not in sys.path:
    sys.path.insert(0, _REPO)

import concourse.bacc as bacc  # noqa: E402
import concourse.mybir as mybir  # noqa: E402
from concourse import tile  # noqa: E402
from concourse.bass_utils import run_bass_kernel_spmd  # noqa: E402

import ml_dtypes  # noqa: E402

BF16 = ml_dtypes.bfloat16

N_CORES = 8
D = 128
NQ = 5  # q = relu(h3) @ (W4*alpha), 5 cols
TILE_ROWS = 1024
BLOCKS_PER_TILE = TILE_ROWS // 128  # 8
CHUNK_TILES = 4  # stats chunk = 4096 rows
GROUPS_PER_CHUNK = CHUNK_TILES * BLOCKS_PER_TILE  # 32
STATS_W = GROUPS_PER_CHUNK * NQ  # 160
ACT_COLS = 864  # columns of the relu done on ScalarE (rest on DVE)
WARMUP_MMS = 28
TILES_PER_DMA = 4


def build_kernel(rows_per_core: int, act_cols: int = ACT_COLS,
                 warmup: int = WARMUP_MMS):
    """Build the per-core Bacc graph. rows_per_core must divide into chunks."""
    assert rows_per_core % (TILE_ROWS * CHUNK_TILES) == 0
    n_chunks = rows_per_core // (TILE_ROWS * CHUNK_TILES)
    n_tiles = rows_per_core // TILE_ROWS

    f32 = mybir.dt.float32
    bf16 = mybir.dt.bfloat16

    nc = bacc.Bacc("TRN2", target_bir_lowering=False, debug=False,
                   num_devices=N_CORES)

    x_ext = nc.declare_dram_parameter("x", [rows_per_core, D], f32,
                                      isOutput=False)
    # packed bf16 consts: [wc | identity | w4a]
    cb_ext = nc.declare_dram_parameter("cb", [D, 2 * D + NQ], bf16,
                                       isOutput=False)
    bc_ext = nc.declare_dram_parameter("bc", [D, 1], f32, isOutput=False)
    out_ext = nc.declare_dram_parameter("out", [D, 2 * STATS_W], f32,
                                        isOutput=True)

    # DMA granule: partition p holds 32 *consecutive* rows (16 KB contiguous
    # per partition -> 8 KB bf16 write packets); one DMA feeds four compute
    # tiles. Row order within a tile is permuted vs. DRAM, which is fine: the
    # softmax pooling is row-permutation invariant.
    DMA_BLOCKS = TILES_PER_DMA * BLOCKS_PER_TILE
    x_r = x_ext.ap().rearrange("(g p k) f -> g p k f", p=128, k=DMA_BLOCKS)
    n_granules = n_tiles // TILES_PER_DMA

    with tile.TileContext(nc) as tc:
        with (
            tc.tile_pool(name="consts", bufs=1) as cpool,
            tc.tile_pool(name="xf32", bufs=1) as fpool,
            tc.tile_pool(name="xin", bufs=5) as xpool,
            tc.tile_pool(name="xt", bufs=6) as xtpool,
            tc.tile_pool(name="relu", bufs=6) as rpool,
            tc.tile_pool(name="stats", bufs=4) as spool,
            tc.tile_pool(name="acc", bufs=1) as apool,
            tc.tile_pool(name="ps_xt", bufs=2, space="PSUM") as ps_xt,
            tc.tile_pool(name="ps_h3", bufs=2, space="PSUM") as ps_h3,
            tc.tile_pool(name="ps_pq", bufs=2, space="PSUM") as ps_pq,
        ):
            cb_sb = cpool.tile([D, 2 * D + NQ], bf16)
            nc.scalar.dma_start(out=cb_sb[:], in_=cb_ext[:])
            wc_sb = cb_sb[:, 0:D]
            ident_sb = cb_sb[:, D:2 * D]
            w4a_sb = cb_sb[:, 2 * D:2 * D + NQ]
            bc_sb = cpool.tile([D, 1], f32)
            nc.scalar.dma_start(out=bc_sb[:], in_=bc_ext[:])

            # acc layout: [e (160) | qe (160)]
            acc = apool.tile([D, 2 * STATS_W], f32)
            nc.vector.memset(acc[:], 0.0)

            # PE warmup: matmuls on zeroed scratch keep the HAM activity
            # window busy during the DMA ramp so real tiles run at 2.4 GHz.
            if warmup > 0:
                wu_sb = cpool.tile([D, D], bf16)
                nc.vector.memset(wu_sb[:], 0.0)
                # share the pq slot rotation -> no extra PSUM bank
                wu_ps = ps_pq.tile([D, D], f32, tag="pq")
                for _ in range(warmup):
                    nc.tensor.matmul(wu_ps[:], wu_sb[:], wu_sb[:],
                                     start=True, stop=True)

            x_dma = None
            for chunk in range(n_chunks):
                pq = ps_pq.tile([D, STATS_W], f32, tag="pq")
                for t in range(CHUNK_TILES):
                    g_tile = chunk * CHUNK_TILES + t
                    sub = g_tile % TILES_PER_DMA
                    granule = g_tile // TILES_PER_DMA
                    if sub == 0:
                        x_dma = xpool.tile([D, TILES_PER_DMA * TILE_ROWS],
                                           bf16, tag="x_dma")
                        xg = x_r[granule]
                        if granule == 0:
                            # head granule via HWDGE (sync queue) as raw f32
                            # + DVE casts: starts streaming concurrently with
                            # the SWDGE stream and lands tiles earlier
                            xf = fpool.tile([D, TILES_PER_DMA * TILE_ROWS],
                                            f32)
                            for sg in range(TILES_PER_DMA):
                                sl = slice(sg * TILE_ROWS,
                                           (sg + 1) * TILE_ROWS)
                                nc.sync.dma_start(
                                    out=xf[:, sl],
                                    in_=xg[:, sg * BLOCKS_PER_TILE:
                                           (sg + 1) * BLOCKS_PER_TILE],
                                )
                                nc.vector.tensor_copy(x_dma[:, sl],
                                                      xf[:, sl])
                        elif granule == 1 or granule == n_granules - 1:
                            # per-tile SWDGE cast DMAs: finer arrivals at
                            # ramp head and de-quantized tail
                            for sg in range(TILES_PER_DMA):
                                sl = slice(sg * TILE_ROWS,
                                           (sg + 1) * TILE_ROWS)
                                nc.gpsimd.dma_start(
                                    out=x_dma[:, sl],
                                    in_=xg[:, sg * BLOCKS_PER_TILE:
                                           (sg + 1) * BLOCKS_PER_TILE],
                                )
                        else:
                            # SWDGE cast DMA: f32 HBM -> bf16 SBUF
                            nc.gpsimd.dma_start(out=x_dma[:], in_=xg)
                    x_sb = x_dma[:, sub * TILE_ROWS:(sub + 1) * TILE_ROWS]

                    xt_ps = ps_xt.tile([D, TILE_ROWS], bf16)
                    for k in range(BLOCKS_PER_TILE):
                        nc.tensor.transpose(
                            xt_ps[:, 128 * k:128 * (k + 1)],
                            x_sb[:, 128 * k:128 * (k + 1)],
                            ident_sb[:],
                        )
                    xt_sb = xtpool.tile([D, TILE_ROWS], bf16)
                    nc.vector.tensor_copy(xt_sb[:], xt_ps[:])

                    h3_ps = ps_h3.tile([D, TILE_ROWS], f32)
                    for half in range(TILE_ROWS // 512):
                        nc.tensor.matmul(
                            h3_ps[:, 512 * half:512 * (half + 1)],
                            wc_sb[:],
                            xt_sb[:, 512 * half:512 * (half + 1)],
                            start=True, stop=True)

                    relu_sb = rpool.tile([D, TILE_ROWS], bf16)
                    a = min(act_cols, TILE_ROWS)
                    if a > 0:
                        nc.scalar.activation(
                            relu_sb[:, 0:a], h3_ps[:, 0:a],
                            mybir.ActivationFunctionType.Relu,
                            bias=bc_sb[:, 0:1], scale=1.0,
                        )
                    if a < TILE_ROWS:
                        nc.vector.tensor_scalar(
                            relu_sb[:, a:TILE_ROWS], h3_ps[:, a:TILE_ROWS],
                            bc_sb[:, 0:1], 0.0,
                            mybir.AluOpType.add, mybir.AluOpType.max,
                        )

                    for k in range(BLOCKS_PER_TILE):
                        g = t * BLOCKS_PER_TILE + k
                        nc.tensor.matmul(
                            pq[:, NQ * g:NQ * (g + 1)],
                            relu_sb[:, 128 * k:128 * (k + 1)],
                            w4a_sb[:],
                            start=True, stop=True,
                        )
                # stats: e = exp(q) on ACT, qe = q*e on DVE, acc += [e|qe]
                # on GPSIMD (SBUF-only op, keeps ACT/DVE free for relu/evac).
                # Last chunk: split in halves + add on DVE to shorten the
                # serial drain after the final tile.
                st = spool.tile([D, 2 * STATS_W], f32)
                last = chunk == n_chunks - 1
                HW = STATS_W // 2
                for hf in ([0, 1] if last else [0]):
                    w = HW if last else STATS_W
                    pqh = pq[:, hf * HW:hf * HW + w]
                    eh = st[:, hf * HW:hf * HW + w]
                    qeh = st[:, STATS_W + hf * HW:STATS_W + hf * HW + w]
                    nc.scalar.activation(eh, pqh,
                                         mybir.ActivationFunctionType.Exp)
                    nc.vector.tensor_mul(qeh, pqh, eh)
                    if last:
                        nc.vector.tensor_add(
                            acc[:, hf * HW:hf * HW + w],
                            acc[:, hf * HW:hf * HW + w], eh)
                        nc.vector.tensor_add(
                            acc[:, STATS_W + hf * HW:STATS_W + hf * HW + w],
                            acc[:, STATS_W + hf * HW:STATS_W + hf * HW + w],
                            qeh)
                if not last:
                    nc.gpsimd.tensor_add(acc[:], acc[:], st[:])

            nc.sync.dma_start(out=out_ext[:], in_=acc[:])

    nc.compile()
    return nc


def _prep_consts(W1, b1, W2, b2, W3, b3, W4, b4, alpha):
    Wc = (W1.astype(np.float64) @ W2.astype(np.float64)
          @ W3.astype(np.float64))
    bc = ((b1.astype(np.float64) @ W2.astype(np.float64)
           + b2.astype(np.float64)) @ W3.astype(np.float64)
          + b3.astype(np.float64))
    W4a = W4.astype(np.float64) * alpha.astype(np.float64)[None, :]
    return (
        Wc.astype(BF16),
        bc.astype(np.float32).reshape(D, 1),
        W4a.astype(BF16),
    )


_CACHE = {}


def _get_nc(rows_per_core):
    key = rows_per_core
    if key not in _CACHE:
        _CACHE[key] = build_kernel(rows_per_core)
    return _CACHE[key]


def make_in_maps(x, W1, b1, W2, b2, W3, b3, W4, b4, alpha):
    x = np.asarray(x)
    n_total = x.shape[1]
    rows_per_core = n_total // N_CORES
    wc_bf, bc_f32, w4a_bf = _prep_consts(
        np.asarray(W1), np.asarray(b1), np.asarray(W2), np.asarray(b2),
        np.asarray(W3), np.asarray(b3), np.asarray(W4), np.asarray(b4),
        np.asarray(alpha))
    # packed bf16 const block: [wc | identity | w4a]
    cb = np.zeros((D, 2 * D + NQ), dtype=BF16)
    cb[:, 0:D] = wc_bf
    cb[:, D:2 * D] = np.eye(D, dtype=BF16)
    cb[:, 2 * D:2 * D + NQ] = w4a_bf

    xs = np.ascontiguousarray(x.reshape(n_total, D))
    in_maps = []
    for c in range(N_CORES):
        in_maps.append({
            "x": xs[c * rows_per_core:(c + 1) * rows_per_core],
            "cb": cb,
            "bc": bc_f32,
        })
    return in_maps, rows_per_core


def _reference_host(x, W1, b1, W2, b2, W3, b3, W4, b4, alpha):
    """Exact fallback (used only if some alpha[j] == 0)."""
    x64 = np.asarray(x, np.float64).reshape(-1, D)
    h = x64 @ np.asarray(W1, np.float64) + np.asarray(b1, np.float64)
    h = h @ np.asarray(W2, np.float64) + np.asarray(b2, np.float64)
    h = h @ np.asarray(W3, np.float64) + np.asarray(b3, np.float64)
    h = np.maximum(h, 0.0)
    p = h @ np.asarray(W4, np.float64) + np.asarray(b4, np.float64)
    q = np.asarray(alpha, np.float64) * p
    q -= q.max(axis=0, keepdims=True)
    w = np.exp(q)
    w /= w.sum(axis=0, keepdims=True)
    return (p * w).sum(axis=0)[None, :].astype(np.float32)


def run(inputs, trace=False, **run_kwargs):
    """Run the kernel; returns (full_output, BassKernelResults)."""
    in_maps, rows_per_core = make_in_maps(**inputs)
    nc = _get_nc(rows_per_core)
    try:
        res = run_bass_kernel_spmd(nc, in_maps, list(range(N_CORES)),
                                   trace=trace, **run_kwargs)
    except Exception:
        # one retry for transient device errors
        res = run_bass_kernel_spmd(nc, in_maps, list(range(N_CORES)),
                                   trace=trace, **run_kwargs)
    out = _finish(res.results, np.asarray(inputs["alpha"]),
                  np.asarray(inputs["b4"]))
    return out, res


def kernel(x, W1, b1, W2, b2, W3, b3, W4, b4, alpha):
    alpha = np.asarray(alpha)
    if np.any(alpha == 0.0):
        return _reference_host(x, W1, b1, W2, b2, W3, b3, W4, b4, alpha)
    out, _ = run(dict(x=x, W1=W1, b1=b1, W2=W2, b2=b2, W3=W3, b3=b3,
                      W4=W4, b4=b4, alpha=alpha))
    return out


def _finish(results, alpha, b4):
    S = np.zeros((D, 2 * STATS_W), dtype=np.float64)
    for r in results:
        S += r["out"].astype(np.float64)
    # acc layout: [e (160) | qe (160)], each [groups, 5]
    se = S[:, :STATS_W].reshape(D, GROUPS_PER_CHUNK, NQ).sum(axis=(0, 1))
    sqe = S[:, STATS_W:].reshape(D, GROUPS_PER_CHUNK, NQ).sum(axis=(0, 1))
    out = sqe / (alpha.astype(np.float64) * se) + b4.astype(np.float64)
    return out[None, :].astype(np.float32)
